# revision 38
# baseline (speedup 1.0000x reference)
"""Trainium2 Bass kernel for CausalHolographicQKV.

Math: unit_projection/bind/unbind are all pointwise in the Fourier domain
along D, so the FFTs fold into the Q/K/V projection weights (host-side DFT
of the weight matrices).  Real-input conjugate symmetry packs the spectrum
into 1024 real channels: [Re F_0..511 | Im F_0(=0) Im F_1..511].  The
Nyquist bin (Re F_512) rides the spare Im F_0 slot: its bind products are
injected into partition 0 of the Im-tile scan (that lane scans zeros
otherwise), the pair AllReduce carries its half-total automatically, and
its inverse-DFT row replaces the all-zero row 512 of G.  On device:

  forward:  f = What^T @ xT   (PE, channel-major [ch, s])
  bind:     p = (fk * fv) / (|fk||fv|)   (ACT/DVE elementwise, bf16)
  cumsum:   tensor_tensor_scan along the free (s) axis (DVE, fp32 state)
  unbind:   z = (s + offset) * conj(fq)/|fq|
  inverse:  out = zT @ G   (bf16 matmuls; G = packed inverse rDFT)

Single fused pass over sequence blocks computes the batched [3,512]
Nyquist chain, k/v forward + bind + scan, and the q forward (f32r for
phase accuracy; normalized q stashed in SBUF).  The pair AllReduce for
the causal half-offset is issued right after the last scan so it overlaps
the final q block; the unbind+inverse pass runs after it.

Sharding: core c = 2*b + h handles batch b, sequence half h (2048 rows).
The causal cumsum crosses the half boundary only through the total sum of
the first half -- a [128,16] per-pair AllReduce (masked so h0 contributes,
h1 consumes).
"""

import os
import sys

sys.path.insert(0, "/opt/trn_rl_repo")

import ml_dtypes
import numpy as np

import concourse.bacc as bacc
import concourse.mybir as mybir
import concourse.tile as tile
from concourse.bass_utils import run_bass_kernel_spmd

F32 = mybir.dt.float32
F32R = mybir.dt.float32r
BF16 = mybir.dt.bfloat16
F16 = mybir.dt.float16
AO = mybir.AluOpType
ACT = mybir.ActivationFunctionType
NPBF16 = ml_dtypes.bfloat16
NPF16 = np.float16

B, S, D = 4, 4096, 1024
NCORES = 8
SC = S // 2          # rows per core
NB = SC // 512       # s-blocks per core
SBK = 512            # s-block size
KT = 8               # k (contraction) tiles of 128
CT = 8               # channel tiles (0..3 re, 4..7 im)
TINY = 1e-12


def _pack_weight(W):
    # einsum('bsd,ed->bse', x, W) = x @ W.T ; spectrum of that along e.
    Wh = np.fft.rfft(np.asarray(W, np.float64).T, axis=1)  # [D, 513]
    Wt = np.zeros((D, D), np.float64)
    Wt[:, 0:512] = Wh.real[:, 0:512]
    Wt[:, 513:1024] = Wh.imag[:, 1:512]
    return Wt.astype(np.float32), Wh.real[:, 512].astype(np.float32)


def _pack_bias(b):
    bh = np.fft.rfft(np.asarray(b, np.float64))
    bt = np.zeros(D, np.float64)
    bt[0:512] = bh.real[0:512]
    bt[513:1024] = bh.imag[1:512]
    return bt.astype(np.float32), float(bh.real[512])


def _build_G():
    d = np.arange(D)
    G = np.zeros((D, D), np.float64)
    c = np.arange(512)[:, None]
    G[0:512] = (2.0 / D) * np.cos(2 * np.pi * c * d / D)
    G[0] *= 0.5
    G[512:1024] = -(2.0 / D) * np.sin(2 * np.pi * c * d / D)
    # row 512 (the Im_0 slot) is identically zero in the packing; use it for
    # the Nyquist inverse row so z_ny needs no separate matmul.
    G[512] = (1.0 / D) * np.cos(np.pi * d)
    return G.astype(np.float32)


def _slab(Wt, ct):
    """[128 part (contraction chunk within kt), KT*128] slab for ctile ct."""
    W4 = Wt.reshape(KT, 128, CT, 128)  # [kt, p, ct, c]
    return np.ascontiguousarray(
        W4[:, :, ct, :].transpose(1, 0, 2).reshape(128, KT * 128))


def _build_nc(collective=True):
    nc = bacc.Bacc("TRN2", target_bir_lowering=False, debug=False,
                   num_devices=NCORES)

    xth = nc.dram_tensor("xth", [NB, 128, KT * SBK], F16,
                         kind="ExternalInput")
    xt = nc.dram_tensor("xt", [NB, 128, KT * SBK], F32R, kind="ExternalInput")
    wkv = nc.dram_tensor("wkv", [128, 16 * KT * 128], F16,
                         kind="ExternalInput")
    wq = nc.dram_tensor("wq", [128, 8 * KT * 128], F32R, kind="ExternalInput")
    g = nc.dram_tensor("g", [128, CT * D], F16, kind="ExternalInput")
    wny = nc.dram_tensor("wny", [128, KT * 3], F32R, kind="ExternalInput")
    bias = nc.dram_tensor("bias", [128, 28], F32, kind="ExternalInput")
    masks = nc.dram_tensor("masks", [128, 2], F32, kind="ExternalInput")
    outt = nc.dram_tensor("out", [SC, D], F32, kind="ExternalOutput")

    with tile.TileContext(nc) as tc:
        with (
            tc.tile_pool(name="persist", bufs=1) as pp,
            tc.tile_pool(name="dramcc", bufs=1, space="DRAM") as dcc,
        ):
            bias_t = pp.tile([128, 28], F32)
            nc.scalar.dma_start(bias_t[:], bias[:, :])
            masks_t = pp.tile([128, 2], F32)
            nc.scalar.dma_start(masks_t[:], masks[:, :])
            wny_t = pp.tile([128, KT * 3], F32R)
            nc.scalar.dma_start(wny_t[:], wny[:, :])

            # resident state
            sh = [pp.tile([128, SC], F16, name=f"sh{t}") for t in range(CT)]
            qsr = [pp.tile([128, SC], F16, name=f"qsr{t}") for t in range(4)]
            qsi = [pp.tile([128, SC], F16, name=f"qsi{t}") for t in range(4)]
            qny_row = pp.tile([1, SC], F16)   # sign(q_ny) per position

            contrib = pp.tile([128, 16], F32)
            contribm = pp.tile([128, 16], F32)
            zro = pp.tile([128, 16], F32)
            nc.vector.memset(zro[:], 0.0)
            offs = pp.tile([128, 16], F32)

            # ---------------- phase A: forward + bind + scan + q ----------
            with (
                tc.tile_pool(name="wkvp", bufs=1) as wkvp,
                tc.tile_pool(name="xhp", bufs=1) as xhp,
                tc.tile_pool(name="xsp", bufs=1) as xsp,
                tc.tile_pool(name="wqp", bufs=1) as wqp,
                tc.tile_pool(name="tpA", bufs=1) as tpA,
                tc.tile_pool(name="psA", bufs=5, space="PSUM") as psA,
                tc.tile_pool(name="psQ", bufs=3, space="PSUM") as psQ,
            ):
                wkv_t = wkvp.tile([128, 16 * KT * 128], F16)
                wq_t = wqp.tile([128, 8 * KT * 128], F32R)

                # DMA issue order = consumption order: the first k/v slab
                # group, x block 0, remaining slabs interleaved with the f32r
                # x and q weights; later x blocks prefetch inside the loop.
                nc.sync.dma_start(wkv_t[:, 4096:5120], wkv[:, 4096:5120])
                xh0 = xhp.tile([128, KT * SBK], F16, tag="xh", bufs=2,
                               name="xh0")
                nc.sync.dma_start(xh0[:, 0:2048], xth[0, :, 0:2048])
                nc.sync.dma_start(xh0[:, 2048:4096], xth[0, :, 2048:4096])
                nc.sync.dma_start(wkv_t[:, 5120:8192], wkv[:, 5120:8192])
                nc.sync.dma_start(wkv_t[:, 8192:12288], wkv[:, 8192:12288])
                xb0 = xsp.tile([128, KT * SBK], F32R, tag="xb", bufs=2,
                               name="xb0")
                nc.sync.dma_start(xb0[:], xt[0, :, :])
                nc.sync.dma_start(wkv_t[:, 12288:16384], wkv[:, 12288:16384])
                nc.sync.dma_start(wkv_t[:, 0:4096], wkv[:, 0:4096])
                for h2 in range(2):
                    nc.sync.dma_start(wq_t[:, h2 * 4096:(h2 + 1) * 4096],
                                      wq[:, h2 * 4096:(h2 + 1) * 4096])

                # PE warmup: junk chain on the framework const tile (ready
                # at ~0.5us) so the clock is at full p-state when the first
                # real chain starts.
                cb = nc.const_aps.tensor(1.0, (128, 1), BF16)
                warm_t = psQ.tile([128, SBK], F32, tag="q", name="warm0")
                warm = warm_t[0:1, 0:1]
                NW0 = 60
                for wi in range(NW0):
                    nc.tensor.matmul(warm, lhsT=cb, rhs=cb,
                                     start=(wi == 0), stop=(wi == NW0 - 1))

                xh, xb = xh0, xb0
                for blk in range(NB):
                    s0 = blk * SBK

                    snb = vrow = None

                    # k/v channel pairs 1..3 first; the f32r nyquist chain
                    # (which gates only tile pair 0's Im scan) runs once the
                    # f32r x block has streamed in, then pair 0 closes out.
                    for tp in (1, 2, 3, 0):
                        if tp == 0:
                            # --- batched nyquist fwd: [3 ch, 512 pos] ---
                            pnyt = psQ.tile([128, SBK], F32, tag="q",
                                            name=f"pny{blk}")
                            pny = pnyt[0:3, :]
                            for kt in range(KT):
                                nc.tensor.matmul(
                                    pny,
                                    lhsT=wny_t[:, kt * 3:(kt + 1) * 3],
                                    rhs=xb[:, kt * SBK:(kt + 1) * SBK],
                                    start=(kt == 0), stop=(kt == KT - 1))
                            snb = tpA.tile([3, SBK], F16, tag="snb", bufs=1,
                                           name=f"snb{blk}")
                            nc.scalar.activation(snb[:], pny, ACT.Sign,
                                                 bias=bias_t[0:3, 27:28],
                                                 scale=1.0)
                            nc.sync.dma_start(qny_row[0:1, s0:s0 + SBK],
                                              snb[2:3, :])
                            vrow = tpA.tile([1, SBK], F16, tag="vrow", bufs=1,
                                            name=f"vrow{blk}")
                            nc.sync.dma_start(vrow[0:1, :], snb[1:2, :])
                        pk_re = psA.tile([128, SBK], F32, tag="fwd")
                        pk_im = psA.tile([128, SBK], F32, tag="fwd")
                        pv_re = psA.tile([128, SBK], F32, tag="fwd")
                        pv_im = psA.tile([128, SBK], F32, tag="fwd")
                        for gi, psum_t in enumerate((pk_re, pk_im,
                                                     pv_re, pv_im)):
                            base = (tp * 4 + gi) * 1024
                            for kt in range(KT):
                                nc.tensor.matmul(
                                    psum_t[:],
                                    lhsT=wkv_t[:, base + kt * 128:
                                               base + (kt + 1) * 128],
                                    rhs=xh[:, kt * SBK:(kt + 1) * SBK],
                                    start=(kt == 0), stop=(kt == KT - 1))

                        kre = tpA.tile([128, SBK], F16, tag="kre", bufs=2)
                        nc.scalar.activation(kre[:], pk_re[:], ACT.Identity,
                                             bias=bias_t[:, tp:tp + 1],
                                             scale=1.0)
                        kim = tpA.tile([128, SBK], F16, tag="kim", bufs=2)
                        nc.scalar.activation(kim[:], pk_im[:], ACT.Identity,
                                             bias=bias_t[:, 4 + tp:5 + tp],
                                             scale=1.0)
                        vre = tpA.tile([128, SBK], F16, tag="vre", bufs=2)
                        nc.scalar.activation(vre[:], pv_re[:], ACT.Identity,
                                             bias=bias_t[:, 8 + tp:9 + tp],
                                             scale=1.0)
                        vim = tpA.tile([128, SBK], F16, tag="vim", bufs=2)
                        nc.scalar.activation(vim[:], pv_im[:], ACT.Identity,
                                             bias=bias_t[:, 12 + tp:13 + tp],
                                             scale=1.0)

                        sqa = tpA.tile([128, SBK], F16, tag="sqa", bufs=1)
                        nc.vector.tensor_mul(sqa[:], kre[:], kre[:])
                        sqb = tpA.tile([128, SBK], F16, tag="sqb", bufs=1)
                        nc.vector.tensor_mul(sqb[:], kim[:], kim[:])
                        kk = tpA.tile([128, SBK], F16, tag="kk", bufs=1)
                        nc.vector.tensor_add(kk[:], sqa[:], sqb[:])
                        sqc = tpA.tile([128, SBK], F16, tag="sqa", bufs=1,
                                       name=f"sqc_{blk}_{tp}")
                        nc.vector.tensor_mul(sqc[:], vre[:], vre[:])
                        sqd = tpA.tile([128, SBK], F16, tag="sqb", bufs=1,
                                       name=f"sqd_{blk}_{tp}")
                        nc.vector.tensor_mul(sqd[:], vim[:], vim[:])
                        vv = tpA.tile([128, SBK], F16, tag="vv", bufs=1,
                                      name=f"vv_{blk}_{tp}")
                        nc.vector.tensor_add(vv[:], sqc[:], sqd[:])
                        mm = tpA.tile([128, SBK], F32, tag="mm", bufs=1)
                        nc.vector.scalar_tensor_tensor(
                            out=mm[:], in0=kk[:], scalar=TINY, in1=vv[:],
                            op0=AO.add, op1=AO.mult)
                        r2 = tpA.tile([128, SBK], F32, tag="r2", bufs=1)
                        nc.vector.reciprocal_approx_fast(out=r2[:], in_=mm[:])
                        rkv = tpA.tile([128, SBK], F16, tag="rkv", bufs=2)
                        nc.scalar.activation(rkv[:], r2[:], ACT.Sqrt)

                        krn = tpA.tile([128, SBK], F16, tag="krn", bufs=1)
                        nc.vector.tensor_mul(krn[:], kre[:], rkv[:])
                        kin = tpA.tile([128, SBK], F16, tag="kin", bufs=1)
                        nc.vector.tensor_mul(kin[:], kim[:], rkv[:])

                        t1 = tpA.tile([128, SBK], F16, tag="t1", bufs=1)
                        nc.vector.tensor_mul(t1[:], krn[:], vre[:])
                        t2 = tpA.tile([128, SBK], F16, tag="t2", bufs=1)
                        nc.vector.tensor_mul(t2[:], kin[:], vim[:])
                        t3 = tpA.tile([128, SBK], F16, tag="t3", bufs=1)
                        nc.vector.tensor_mul(t3[:], krn[:], vim[:])
                        t4 = tpA.tile([128, SBK], F16, tag="t4", bufs=1)
                        nc.vector.tensor_mul(t4[:], kin[:], vre[:])

                        if tp == 0:
                            # nyquist bind products ride the Im_0 lane
                            nc.vector.tensor_mul(t3[0:1, :], snb[0:1, :],
                                                 vrow[0:1, :])

                        init_re = 0.0 if blk == 0 else sh[tp][:, s0 - 1:s0]
                        nc.vector.tensor_tensor_scan(
                            out=sh[tp][:, s0:s0 + SBK], data0=t1[:],
                            data1=t2[:], initial=init_re,
                            op0=AO.add, op1=AO.subtract)
                        init_im = 0.0 if blk == 0 else sh[tp + 4][:, s0 - 1:s0]
                        nc.vector.tensor_tensor_scan(
                            out=sh[tp + 4][:, s0:s0 + SBK], data0=t3[:],
                            data1=t4[:], initial=init_im,
                            op0=AO.add, op1=AO.add)

                    if blk == NB - 1:
                        # totals + pairwise collective; issued here so the
                        # transfer overlaps the last q block.
                        nc.vector.memset(contrib[:], 0.0)
                        for t in range(CT):
                            nc.vector.tensor_copy(contrib[:, t:t + 1],
                                                  sh[t][:, SC - 1:SC])
                        nc.vector.tensor_scalar_mul(contribm[:], contrib[:],
                                                    masks_t[:, 0:1])
                        # pair ReduceScatter: rank h receives half h of the
                        # payload, so h0 lands on zeros and h1 on h0's total
                        # -- the consumer-side mask costs no compute op.
                        cc_in = dcc.tile([256, 16], F32)
                        cc_out = dcc.tile([128, 16], F32)
                        nc.sync.dma_start(cc_in[0:128, :], zro[:])
                        nc.sync.dma_start(cc_in[128:256, :], contribm[:])
                        if collective:
                            nc.gpsimd.collective_compute(
                                "ReduceScatter", AO.add,
                                replica_groups=[[0, 1], [2, 3], [4, 5],
                                                [6, 7]],
                                ins=[cc_in[:].opt()], outs=[cc_out[:].opt()])
                        else:
                            nc.sync.dma_start(cc_out[:], cc_in[128:256, :])
                        nc.sync.dma_start(offs[:], cc_out[:])

                    # prefetch the next block's x before the q work so the
                    # transfers overlap this block's q chains.
                    if blk + 1 < NB:
                        xh_n = xhp.tile([128, KT * SBK], F16, tag="xh",
                                        bufs=2, name=f"xh{blk + 1}")
                        nc.sync.dma_start(xh_n[:], xth[blk + 1, :, :])
                        xb_n = xsp.tile([128, KT * SBK], F32R, tag="xb",
                                        bufs=2, name=f"xb{blk + 1}")
                        nc.sync.dma_start(xb_n[:], xt[blk + 1, :, :])

                    # --- q forward + unit projection (stash phases) ---
                    for tp in range(4):
                        pq_re = psQ.tile([128, SBK], F32, tag="q")
                        pq_im = psQ.tile([128, SBK], F32, tag="q")
                        for gi, psum_t in enumerate((pq_re, pq_im)):
                            base = (tp * 2 + gi) * 1024
                            for kt in range(KT):
                                nc.tensor.matmul(
                                    psum_t[:],
                                    lhsT=wq_t[:, base + kt * 128:
                                              base + (kt + 1) * 128],
                                    rhs=xb[:, kt * SBK:(kt + 1) * SBK],
                                    start=(kt == 0), stop=(kt == KT - 1))

                        qre = tpA.tile([128, SBK], F16, tag="kre", bufs=2,
                                       name=f"qre_{blk}_{tp}")
                        nc.scalar.activation(qre[:], pq_re[:], ACT.Identity,
                                             bias=bias_t[:, 16 + tp:17 + tp],
                                             scale=1.0)
                        qim = tpA.tile([128, SBK], F16, tag="kim", bufs=2,
                                       name=f"qim_{blk}_{tp}")
                        nc.scalar.activation(qim[:], pq_im[:], ACT.Identity,
                                             bias=bias_t[:, 20 + tp:21 + tp],
                                             scale=1.0)
                        sqa = tpA.tile([128, SBK], F16, tag="sqa", bufs=1,
                                       name=f"qs1_{blk}_{tp}")
                        nc.vector.tensor_mul(sqa[:], qre[:], qre[:])
                        sqb = tpA.tile([128, SBK], F16, tag="sqb", bufs=1,
                                       name=f"qs2_{blk}_{tp}")
                        nc.vector.tensor_mul(sqb[:], qim[:], qim[:])
                        qq = tpA.tile([128, SBK], F32, tag="mm", bufs=1,
                                      name=f"qq_{blk}_{tp}")
                        nc.vector.scalar_tensor_tensor(
                            out=qq[:], in0=sqa[:], scalar=TINY, in1=sqb[:],
                            op0=AO.add, op1=AO.add)
                        r2q = tpA.tile([128, SBK], F32, tag="r2", bufs=1,
                                       name=f"r2q_{blk}_{tp}")
                        nc.vector.reciprocal_approx_fast(out=r2q[:], in_=qq[:])
                        rq = tpA.tile([128, SBK], F16, tag="rkv", bufs=2,
                                      name=f"rq_{blk}_{tp}")
                        nc.scalar.activation(rq[:], r2q[:], ACT.Sqrt)
                        nc.vector.tensor_mul(qsr[tp][:, s0:s0 + SBK],
                                             qre[:], rq[:])
                        nc.vector.tensor_mul(qsi[tp][:, s0:s0 + SBK],
                                             qim[:], rq[:])

                    if blk + 1 < NB:
                        xh, xb = xh_n, xb_n



            # ---------------- phase B: unbind + inverse --------------------
            with (
                tc.tile_pool(name="gp", bufs=1) as gp,
                tc.tile_pool(name="tpB", bufs=1) as tpB,
                tc.tile_pool(name="zp", bufs=1) as zp,
                tc.tile_pool(name="obp", bufs=2) as obp,
                tc.tile_pool(name="psI", bufs=4, space="PSUM") as psI,
            ):
                g_t = gp.tile([128, CT * D], F16)
                for h2 in range(2):
                    nc.sync.dma_start(g_t[:, h2 * 4096:(h2 + 1) * 4096],
                                      g[:, h2 * 4096:(h2 + 1) * 4096])

                # PE warmup across the phase handoff: keep the clock hot
                # while the first unbind tiles drain through ACT/DVE.
                warmb_t = psI.tile([128, 512], F32, tag="inv", name="warmb")
                warmb = warmb_t[:, 0:64]
                NWB = 150
                for wi in range(NWB):
                    nc.tensor.matmul(warmb, lhsT=g_t[:, 0:128],
                                     rhs=g_t[:, 0:64],
                                     start=(wi == 0), stop=(wi == NWB - 1))

                for blk in range(NB):
                    s0 = blk * SBK
                    zts = [zp.tile([128, SBK], F16, tag=f"z{t}", bufs=2,
                                   name=f"z{t}_{blk}")
                           for t in range(CT)]
                    for tp in range(4):
                        o_re = offs[:, tp:tp + 1]
                        o_im = offs[:, 4 + tp:5 + tp]
                        sre = tpB.tile([128, SBK], F16, tag="sre", bufs=2)
                        nc.scalar.activation(sre[:], sh[tp][:, s0:s0 + SBK],
                                             ACT.Identity, bias=o_re,
                                             scale=1.0)
                        sim = tpB.tile([128, SBK], F16, tag="sim", bufs=2)
                        nc.scalar.activation(sim[:],
                                             sh[tp + 4][:, s0:s0 + SBK],
                                             ACT.Identity, bias=o_im,
                                             scale=1.0)

                        t5 = tpB.tile([128, SBK], F16, tag="t5", bufs=1)
                        nc.vector.tensor_mul(t5[:], sre[:],
                                             qsr[tp][:, s0:s0 + SBK])
                        t6 = tpB.tile([128, SBK], F16, tag="t6", bufs=1)
                        nc.vector.tensor_mul(t6[:], sim[:],
                                             qsi[tp][:, s0:s0 + SBK])
                        t7 = tpB.tile([128, SBK], F16, tag="t7", bufs=1)
                        nc.vector.tensor_mul(t7[:], sim[:],
                                             qsr[tp][:, s0:s0 + SBK])
                        t8 = tpB.tile([128, SBK], F16, tag="t8", bufs=1)
                        nc.vector.tensor_mul(t8[:], sre[:],
                                             qsi[tp][:, s0:s0 + SBK])
                        nc.vector.tensor_add(zts[tp][:], t5[:], t6[:])
                        nc.vector.tensor_sub(zts[tp + 4][:], t7[:], t8[:])

                        if tp == 0:
                            # fix the ny lane: z_ny = (s_ny+o_ny)*sign(q_ny);
                            # computed value has sign(q_DC) instead.
                            crow = tpB.tile([1, SBK], F16, tag="crow",
                                            bufs=2, name=f"crow_{blk}")
                            nc.vector.tensor_mul(
                                crow[:], qny_row[0:1, s0:s0 + SBK],
                                qsr[0][0:1, s0:s0 + SBK])
                            zrow = tpB.tile([1, SBK], F16, tag="zrow",
                                            bufs=2, name=f"zrow_{blk}")
                            nc.vector.tensor_mul(zrow[:], zts[4][0:1, :],
                                                 crow[:])
                            nc.vector.tensor_copy(zts[4][0:1, :], zrow[:])

                    ct_order = (0, 4, 1, 5, 2, 6, 3, 7)  # zts ready order
                    for ss in range(4):
                        if blk == NB - 1 and ss == 3:
                            # drain the very last rows in quarter chunks so
                            # the final ACT->DMA->sem chain is short.
                            for qtr in range(4):
                                poq = psI.tile([128, 256], F32, tag="invq",
                                               bufs=4,
                                               name=f"poq{qtr}")
                                for ci, ct in enumerate(ct_order):
                                    z128 = zts[ct][:, ss * 128:
                                                   (ss + 1) * 128]
                                    nc.tensor.matmul(
                                        poq[:], lhsT=z128,
                                        rhs=g_t[:, ct * D + qtr * 256:
                                                ct * D + qtr * 256 + 256],
                                        start=(ci == 0),
                                        stop=(ci == CT - 1))
                                obq = obp.tile([128, 256], F32, tag="obq",
                                               bufs=4, name=f"obq{qtr}")
                                nc.scalar.activation(obq[:], poq[:],
                                                     ACT.Copy)
                                nc.sync.dma_start(
                                    outt[s0 + ss * 128:s0 + (ss + 1) * 128,
                                         qtr * 256:(qtr + 1) * 256],
                                    obq[:])
                            continue
                        po0 = psI.tile([128, 512], F32, tag="inv")
                        po1 = psI.tile([128, 512], F32, tag="inv")
                        for ci, ct in enumerate(ct_order):
                            z128 = zts[ct][:, ss * 128:(ss + 1) * 128]
                            nc.tensor.matmul(
                                po0[:], lhsT=z128,
                                rhs=g_t[:, ct * D:ct * D + 512],
                                start=(ci == 0), stop=(ci == CT - 1))
                            nc.tensor.matmul(
                                po1[:], lhsT=z128,
                                rhs=g_t[:, ct * D + 512:ct * D + 1024],
                                start=(ci == 0), stop=(ci == CT - 1))
                        ob = obp.tile([128, D], F32, tag="ob")
                        nc.scalar.activation(ob[:, 0:512], po0[:], ACT.Copy)
                        nc.sync.dma_start(
                            outt[s0 + ss * 128:s0 + (ss + 1) * 128, 0:512],
                            ob[:, 0:512])
                        nc.scalar.activation(ob[:, 512:1024], po1[:],
                                             ACT.Copy)
                        nc.sync.dma_start(
                            outt[s0 + ss * 128:s0 + (ss + 1) * 128,
                                 512:1024],
                            ob[:, 512:1024])

    nc.compile()
    return nc


_NC_CACHE = None


def _prep_in_maps(x, Wq, bq, Wk, bk, Wv, bv):
    x = np.asarray(x, np.float32)
    Wtq, wnyq = _pack_weight(Wq)
    btq, bnyq = _pack_bias(bq)
    Wtk, wnyk = _pack_weight(Wk)
    btk, bnyk = _pack_bias(bk)
    Wtv, wnyv = _pack_weight(Wv)
    btv, bnyv = _pack_bias(bv)
    G = _build_G()

    # kv consumption order: for tp: (k,tp),(k,tp+4),(v,tp),(v,tp+4)
    wkv_h = np.zeros((128, 16 * KT * 128), NPF16)
    gi = 0
    for tp in range(4):
        for Wt, ct in ((Wtk, tp), (Wtk, tp + 4), (Wtv, tp), (Wtv, tp + 4)):
            wkv_h[:, gi * 1024:(gi + 1) * 1024] = _slab(Wt, ct)
            gi += 1
    # q order: tp0re, tp0im, tp1re, tp1im... = ctiles [0,4,1,5,2,6,3,7]
    wq_h = np.zeros((128, 8 * KT * 128), np.float32)
    for gi, ct in enumerate([0, 4, 1, 5, 2, 6, 3, 7]):
        wq_h[:, gi * 1024:(gi + 1) * 1024] = _slab(Wtq, ct)
    g_h = np.ascontiguousarray(
        G.reshape(CT, 128, D).transpose(1, 0, 2).reshape(128, CT * D)
    ).astype(NPF16)
    wny_kpc = np.stack([wnyk, wnyv, wnyq], axis=1).reshape(KT, 128, 3)
    wny_h = np.ascontiguousarray(
        wny_kpc.transpose(1, 0, 2).reshape(128, KT * 3)).astype(np.float32)
    bias_h = np.zeros((128, 28), np.float32)
    for pi, bt in enumerate((btk, btv, btq)):
        bias_h[:, pi * 8:(pi + 1) * 8] = bt.reshape(8, 128).T
    bias_h[0, 27] = bnyk
    bias_h[1, 27] = bnyv
    bias_h[2, 27] = bnyq

    in_maps = []
    for c in range(NCORES):
        b, h = c // 2, c % 2
        xs = np.ascontiguousarray(
            x[b, h * SC:(h + 1) * SC, :].T.reshape(KT, 128, NB, SBK)
            .transpose(2, 1, 0, 3).reshape(NB, 128, KT * SBK)
        ).astype(np.float32)
        m = np.zeros((128, 2), np.float32)
        m[:, 0] = 1.0 if h == 0 else 0.0
        m[:, 1] = 0.0 if h == 0 else 1.0
        in_maps.append(dict(
            xt=xs, xth=xs.astype(NPF16), wkv=wkv_h, wq=wq_h, g=g_h,
            wny=wny_h, bias=bias_h, masks=m))
    return in_maps


def kernel(x, Wq, bq, Wk, bk, Wv, bv):
    global _NC_CACHE

    in_maps = _prep_in_maps(x, Wq, bq, Wk, bk, Wv, bv)

    if _NC_CACHE is None:
        _NC_CACHE = _build_nc()
    nc = _NC_CACHE

    trace = bool(int(os.environ.get("KERNEL_TRACE", "0")))
    res = None
    if trace:
        try:
            res = run_bass_kernel_spmd(nc, in_maps, core_ids=list(range(NCORES)),
                                       trace=True)
        except Exception as e:  # ntff hook missing on older axon clients
            print(f"trace unavailable ({e}); rerunning without trace", flush=True)
            res = None
    if res is None:
        res = run_bass_kernel_spmd(nc, in_maps, core_ids=list(range(NCORES)))
    if res.exec_time_ns is not None:
        print(f"HW exec time: {res.exec_time_ns} ns", flush=True)
        kernel.last_exec_time_ns = res.exec_time_ns

    out = np.zeros((B, S, D), np.float32)
    for c in range(NCORES):
        b, h = c // 2, c % 2
        out[b, h * SC:(h + 1) * SC, :] = res.results[c]["out"]
    return out
